# revision 14
# baseline (speedup 1.0000x reference)
"""Trainium (trn2) Bass kernel for a 2-layer GAT over N=100k nodes / E=1.7M edges.

Strategy (v2 — two-pass per layer)
----------------------------------
Edges are sorted by destination on the host (index-only preprocessing); the
destination axis is sharded across the 8 NeuronCores in contiguous 128-node
windows (98 windows per core). Each GAT layer runs as TWO device passes with a
host permutation between them (the host performs no FLOPs — only index
gathers, transposes and constant fills):

* pass A (node level, tiny): h = x @ W plus the folded attention logits
  al_src = h.a_s and al_dst = h.a_d, computed once per NODE (not per edge)
  as a chunked matmul with the x-chunk stationary. Layer 2's pass A also
  applies the inter-layer ELU to its input table on the fly.
* host: gathers per-edge streams from the pass-A node tables: h[src] rows,
  and a narrow z-stream [al_s[src] | al_d[dst] | rel_dst]. Streams are laid
  out in "grouped" order (partition-major within each 16-tile group) so DMA
  rows are >=4KB contiguous.
* pass B (edge level, the hot loop): per 128-edge tile,
    z = al_s + al_d           (DVE, batched per 16-tile group)
    z = leaky_relu(z); e = exp(z - 4)   (Scalar engine, batched; the -4
                                         cancels in the softmax)
    msg = h_src * e           (DVE; features are HEAD-MINOR [c,h] so the
                               per-head broadcast keeps a packed last dim
                               and the 2x 16-bit DVE mode)
    S[e,n] = (iota == rel)    (DVE tensor_scalar is_equal, 4x mode)
    psum[n, :] += S^T @ [msg | e]   (ONE matmul per tile: 128-row weight
                                     load + 136 streamed rows; denominators
                                     ride as extra columns)
  and per 128-node window an epilogue divides by the denominator.
  Layer 2 has a single head, so the exp scale is folded INTO the one-hot
  (tensor_scalar is_equal*mult) and pass B needs no per-edge multiply at
  all: rhs = [h2_src | 1].

Biases are zero in this problem (asserted); if nonzero they'd fold into the
pass-A tables (softmax weights sum to 1). All floating-point math runs on
device; f16 streams/tables, f32 PSUM accumulation.

Environment workarounds: this walrus build allows only ONE semaphore wait per
instruction (split onto nop carriers post-scheduling); no GPSIMD ucode
libraries (no indirect DMA - hence host-gathered streams).
"""
import numpy as np

import concourse.bass as bass
import concourse.mybir as mybir
import concourse.tile as tile
from concourse.bass_utils import run_bass_kernel_spmd

P = 128
F16 = mybir.dt.float16
F32 = mybir.dt.float32
AF = mybir.ActivationFunctionType
OP = mybir.AluOpType
NEG_SLOPE = 0.2
EXP_BIAS = -4.0     # exp(z + EXP_BIAS): constant shift cancels in softmax
GRP = 16            # tiles per stream group
PAD_REL = 255.0     # rel value for pad slots -> is_equal never matches
N_CORES = 8

# ------------------------------------------------------------------ patches

_wsplit_counter = [0]


def _split_excess_waits(nc, max_waits=1):
    """This walrus build rejects >1 sem-wait per instruction ("Too many sync
    wait commands"). Move overflow waits onto same-engine nop carriers."""
    n_split = 0
    for f in nc.m.functions:
        for blk in f.blocks:
            changed = False
            out = []
            for inst in blk.instructions:
                si = inst.sync_info
                if si is not None and len(si.on_wait) > max_waits:
                    waits = list(si.on_wait)
                    keep = waits[len(waits) - max_waits:]
                    overflow = waits[: len(waits) - max_waits]
                    for i in range(0, len(overflow), max_waits):
                        _wsplit_counter[0] += 1
                        nop = mybir.InstNoOp(
                            name=f"I-wsplit-{_wsplit_counter[0]}", ins=[], outs=[])
                        nop.engine = inst.engine
                        nop.sync_info = mybir.SyncInfo(
                            on_wait=overflow[i: i + max_waits], on_update=[])
                        out.append(nop)
                    inst.sync_info = mybir.SyncInfo(
                        on_wait=keep, on_update=list(si.on_update))
                    changed = True
                    n_split += 1
                out.append(inst)
            if changed:
                blk.instructions = out
    return n_split


def _finalize_kernel(nc):
    import bass_rust as _bass_rust
    from concourse.library_config import all_libraries, standard
    from concourse.library_overlay import lower_extended_insts

    inst_type_to_lib_mask = {}
    for lib in all_libraries:
        for inst_type in lib.instructions:
            inst_type_to_lib_mask[inst_type] = inst_type_to_lib_mask.get(
                inst_type, 0) | (1 << lib.index)
    _bass_rust.insert_library_loads(
        nc, inst_type_to_lib_mask, len(all_libraries), standard.index)
    lower_extended_insts(nc)
    _split_excess_waits(nc)


# ------------------------------------------------------------------ host prep

class _Graph:
    """Host-side index preprocessing: sort by dst, shard dst windows across
    cores, pad per-window tile counts to a global schedule so all cores run
    one identical SPMD program. Streams are emitted in grouped layout:
    row r = (g*128 + p)*GRP + j holds edge slot (g*GRP + j)*128 + p."""

    def __init__(self, edge_index, n_nodes, n_cores):
        self.N = n_nodes
        self.C = n_cores
        src = np.asarray(edge_index[0], dtype=np.int64)
        dst = np.asarray(edge_index[1], dtype=np.int64)
        perm = np.argsort(dst, kind="stable")
        src_s = src[perm].astype(np.int32)
        dst_s = dst[perm].astype(np.int32)

        n_win_total = (n_nodes + P - 1) // P
        self.wpc = (n_win_total + n_cores - 1) // n_cores
        self.n_win = self.wpc * n_cores
        self.shard_nodes = self.wpc * P

        bounds = np.searchsorted(dst_s, np.arange(0, self.n_win + 1) * P)
        counts = np.zeros((n_cores, self.wpc), dtype=np.int64)
        for k in range(n_cores):
            for i in range(self.wpc):
                w = k * self.wpc + i
                if w < n_win_total:
                    counts[k, i] = bounds[w + 1] - bounds[w]
        self.PC = np.maximum(np.ceil(counts / P).astype(np.int64).max(axis=0), 1)
        self.T = int(self.PC.sum())
        self.n_groups = (self.T + GRP - 1) // GRP
        self.T_pad = self.n_groups * GRP

        S = self.T_pad * P
        slot_src = np.zeros((n_cores, S), dtype=np.int32)
        slot_dst = np.zeros((n_cores, S), dtype=np.int32)
        slot_rel = np.full((n_cores, S), int(PAD_REL), dtype=np.int32)
        for k in range(n_cores):
            t0 = 0
            for i in range(self.wpc):
                w = k * self.wpc + i
                cnt = int(counts[k, i])
                if cnt > 0:
                    e0 = bounds[w]
                    sl = t0 * P
                    slot_src[k, sl:sl + cnt] = src_s[e0:e0 + cnt]
                    slot_dst[k, sl:sl + cnt] = dst_s[e0:e0 + cnt]
                    slot_rel[k, sl:sl + cnt] = dst_s[e0:e0 + cnt] - w * P
                t0 += int(self.PC[i])

        # grouped-layout permutation: new row r <- old slot s
        order = np.arange(S).reshape(self.n_groups, GRP, P)
        order = order.transpose(0, 2, 1).reshape(-1)
        self.gsrc = slot_src[:, order]
        self.gdst = slot_dst[:, order]
        self.grel = slot_rel[:, order].astype(np.float16)

    def stream_h(self, table, core, extra_ones=False):
        """[n_groups, P, GRP*W] f16 gather of table rows by slot src."""
        W = table.shape[1]
        Wx = W + 2 if extra_ones else W
        out = np.empty((self.T_pad * P, Wx), dtype=np.float16)
        out[:, :W] = table[self.gsrc[core]]
        if extra_ones:
            out[:, W] = 1.0
            out[:, W + 1] = 0.0
        return np.ascontiguousarray(
            out.reshape(self.n_groups, P, GRP * Wx))

    def stream_z(self, als, ald, core):
        """[n_groups, P, GRP*ZC] f16: [al_s[src] | al_d[dst]]."""
        H = als.shape[1]
        ZC = 2 * H
        out = np.empty((self.T_pad * P, ZC), dtype=np.float16)
        out[:, 0:H] = als[self.gsrc[core]]
        out[:, H:2 * H] = ald[self.gdst[core]]
        return np.ascontiguousarray(
            out.reshape(self.n_groups, P, GRP * ZC))

    def stream_rel(self, core):
        return np.ascontiguousarray(
            self.grel[core].astype(np.float32).reshape(
                self.n_groups, P, GRP))


# ------------------------------------------------------------------ builders

def _build_node(c_in, n_chunks, cols, elu_input, bench_loop=1):
    """Pass A: hext[n, :] = (elu?)(x[n]) @ wext for one shard of nodes."""
    nc = bass.Bass()
    xT = nc.dram_tensor("xT", [c_in, n_chunks * P], F16, kind="ExternalInput")
    wext = nc.dram_tensor("wext", [c_in, cols], F16, kind="ExternalInput")
    out = nc.dram_tensor("out", [n_chunks * P, cols], F16, kind="ExternalOutput")
    NB = 4

    with tile.TileContext(nc) as tc:
        with (
            tc.tile_pool(name="const", bufs=1) as constp,
            tc.tile_pool(name="x", bufs=3) as xp,
            tc.tile_pool(name="w", bufs=4) as wp,
            tc.tile_pool(name="ps", bufs=4, space="PSUM") as psp,
        ):
            wext_sb = constp.tile([c_in, cols], F16)
            nc.sync.dma_start(out=wext_sb[:], in_=wext[:])

            def body(_iv=None):
                for c0 in range(0, n_chunks, NB):
                    nb = min(NB, n_chunks - c0)
                    xc = xp.tile([c_in, NB * P], F16, tag="xc")
                    nc.sync.dma_start(out=xc[:, :nb * P],
                                      in_=xT[:, c0 * P:(c0 + nb) * P])
                    if elu_input:
                        # ELU = max(x,0) - 1 + exp(min(x,0))
                        mn = wp.tile([c_in, NB * P], F16, tag="mn")
                        nc.vector.tensor_scalar(
                            mn[:, :nb * P], xc[:, :nb * P], 0.0, None, OP.min)
                        ex = wp.tile([c_in, NB * P], F16, tag="ex")
                        nc.scalar.activation(ex[:, :nb * P], mn[:, :nb * P],
                                             AF.Exp)
                        mx = wp.tile([c_in, NB * P], F16, tag="mx")
                        nc.vector.tensor_scalar(
                            mx[:, :nb * P], xc[:, :nb * P], 0.0, -1.0,
                            OP.max, OP.add)
                        xe = wp.tile([c_in, NB * P], F16, tag="xe")
                        nc.vector.tensor_tensor(
                            out=xe[:, :nb * P], in0=mx[:, :nb * P],
                            in1=ex[:, :nb * P], op=OP.add)
                        xu = xe
                    else:
                        xu = xc
                    for c in range(nb):
                        ps = psp.tile([P, cols], F32, tag="ps")
                        nc.tensor.matmul(ps[:], xu[:, c * P:(c + 1) * P],
                                         wext_sb[:], start=True, stop=True)
                        he = wp.tile([P, cols], F16, tag="he")
                        nc.vector.tensor_copy(he[:], ps[:])
                        nc.scalar.dma_start(
                            out=out[(c0 + c) * P:(c0 + c + 1) * P, :],
                            in_=he[:])

            if bench_loop > 1:
                with tc.For_i(0, bench_loop, 1) as _iv:
                    body(_iv)
            else:
                body()
    _finalize_kernel(nc)
    return nc


def _build_edge(T, PC, wpc, n_groups, HC, heads, hid, out_f32, bench_loop=1):
    """Pass B: edge aggregation. heads>1 -> per-edge msg multiply (layer 1);
    heads==1 -> exp folded into the one-hot (layer 2)."""
    L2 = heads == 1
    CA = HC + heads                 # matmul rhs width (msg|exp / h|1)
    SW = HC + 2 if L2 else HC       # h-stream row width
    ZC = 2 * heads                  # z-stream row width
    nc = bass.Bass()
    hsrc = nc.dram_tensor("hsrc", [n_groups, P, GRP * SW], F16,
                          kind="ExternalInput")
    zstr = nc.dram_tensor("zstr", [n_groups, P, GRP * ZC], F16,
                          kind="ExternalInput")
    rstr = nc.dram_tensor("rstr", [n_groups, P, GRP], F32,
                          kind="ExternalInput")
    iota_c = nc.dram_tensor("iota", [P, P], F16, kind="ExternalInput")
    out = nc.dram_tensor("out", [wpc * P, HC], F32 if out_f32 else F16,
                         kind="ExternalOutput")

    tile_win = []
    for i in range(wpc):
        tile_win += [i] * int(PC[i])
    first_of_win, last_of_win = {}, {}
    for t, w in enumerate(tile_win):
        first_of_win.setdefault(w, t)
        last_of_win[w] = t

    with tile.TileContext(nc) as tc:
        with (
            tc.tile_pool(name="const", bufs=1) as constp,
            tc.tile_pool(name="stream", bufs=3) as streamp,
            tc.tile_pool(name="msg", bufs=3) as msgp,
            tc.tile_pool(name="S", bufs=6) as sp,
            tc.tile_pool(name="work", bufs=3) as workp,
            tc.tile_pool(name="epi", bufs=3) as epip,
            tc.tile_pool(name="psW", bufs=3, space="PSUM") as psW,
        ):
            iota_sb = constp.tile([P, P], F16)
            nc.sync.dma_start(out=iota_sb[:], in_=iota_c[:])
            ebias_sb = constp.tile([P, 1], F32)
            nc.vector.memset(ebias_sb[:], EXP_BIAS)

            def body(_iv=None):
                psw_cur = [None]
                for g in range(n_groups):
                    tlo, thi = g * GRP, min(T, g * GRP + GRP)
                    ng = thi - tlo
                    hs = streamp.tile([P, GRP * SW], F16, tag="hs")
                    nc.sync.dma_start(out=hs[:, :ng * SW],
                                      in_=hsrc[g, :, :ng * SW])
                    zs = streamp.tile([P, GRP * ZC], F16, tag="zs")
                    nc.scalar.dma_start(out=zs[:, :ng * ZC],
                                        in_=zstr[g, :, :ng * ZC])
                    rls = streamp.tile([P, GRP], F32, tag="rls")
                    nc.scalar.dma_start(out=rls[:, :ng],
                                        in_=rstr[g, :, :ng])

                    # z = al_s + al_d  (batched over the group)
                    def ap3(base, d1, d2):
                        return bass.AP(base.tensor, base.offset,
                                       [base.ap[0], d1, d2])

                    za = workp.tile([P, GRP * heads], F16, tag="za")
                    za3 = ap3(za[:], [heads, ng], [1, heads])
                    in0 = ap3(zs[:], [ZC, ng], [1, heads])
                    in1 = ap3(zs[:, heads:heads + 1], [ZC, ng], [1, heads])
                    nc.vector.tensor_tensor(out=za3, in0=in0, in1=in1,
                                            op=OP.add)
                    nc.scalar.activation(za3, za3, AF.Prelu, alpha=NEG_SLOPE)

                    if not L2:
                        msg = msgp.tile([P, GRP * CA], F16, tag="msg")
                        # exp(z-4) into the tail columns of each tile's rhs
                        exp_o = ap3(msg[:, HC:HC + 1], [CA, ng], [1, heads])
                        nc.scalar.activation(exp_o, za3, AF.Exp,
                                             bias=ebias_sb[:])
                        # msg = h_src * exp  (head-minor: packed last dim)
                        def ap4(base, d1, d2, d3):
                            return bass.AP(base.tensor, base.offset,
                                           [base.ap[0], d1, d2, d3])
                        o4 = ap4(msg[:], [CA, ng], [heads, hid], [1, heads])
                        i4 = ap4(hs[:], [SW, ng], [heads, hid], [1, heads])
                        e4 = ap4(msg[:, HC:HC + 1], [CA, ng], [0, hid],
                                 [1, heads])
                        nc.vector.tensor_tensor(out=o4, in0=i4, in1=e4,
                                                op=OP.mult)
                    else:
                        expg = workp.tile([P, GRP], F32, tag="exp")
                        ex_o = ap3(expg[:], [1, ng], [1, 1])
                        nc.scalar.activation(ex_o, za3, AF.Exp,
                                             bias=ebias_sb[:])

                    for j in range(ng):
                        t = tlo + j
                        w = tile_win[t]
                        S_sb = sp.tile([P, P], F16, tag="S")
                        if L2:
                            nc.vector.tensor_scalar(
                                S_sb[:], iota_sb[:], rls[:, j:j + 1],
                                expg[:, j:j + 1], OP.is_equal, OP.mult)
                            rhs = hs[:, j * SW:j * SW + CA]
                        else:
                            nc.vector.tensor_scalar(
                                S_sb[:], iota_sb[:], rls[:, j:j + 1],
                                None, OP.is_equal)
                            rhs = msg[:, j * CA:(j + 1) * CA]
                        if t == first_of_win[w]:
                            psw_t = psW.tile([P, CA], F32, tag="psW")
                            psw_cur[0] = psw_t
                        nc.tensor.matmul(psw_cur[0][:], S_sb[:], rhs,
                                         start=(t == first_of_win[w]),
                                         stop=(t == last_of_win[w]))
                        if t == last_of_win[w]:
                            psw = psw_cur[0]
                            if L2:
                                rec = epip.tile([P, 1], F32, tag="rec")
                                nc.vector.reciprocal(rec[:],
                                                     psw[:, HC:HC + 1])
                                o2 = epip.tile([P, HC],
                                               F32 if out_f32 else F16,
                                               tag="o2")
                                nc.scalar.mul(o2[:], psw[:, 0:HC],
                                              rec[:, 0:1])
                                nc.scalar.dma_start(
                                    out=out[w * P:(w + 1) * P, :], in_=o2[:])
                            else:
                                rec = epip.tile([P, heads], F32, tag="rec")
                                nc.vector.reciprocal(rec[:],
                                                     psw[:, HC:HC + heads])
                                o1 = epip.tile([P, HC], F16, tag="o1")
                                o3 = ap3(o1[:], [heads, hid], [1, heads])
                                p3 = ap3(psw[:], [heads, hid], [1, heads])
                                r3 = ap3(rec[:], [0, hid], [1, heads])
                                nc.vector.tensor_tensor(out=o3, in0=p3,
                                                        in1=r3, op=OP.mult)
                                nc.scalar.dma_start(
                                    out=out[w * P:(w + 1) * P, :], in_=o1[:])

            if bench_loop > 1:
                with tc.For_i(0, bench_loop, 1) as _iv:
                    body(_iv)
            else:
                body()
    _finalize_kernel(nc)
    return nc


# ------------------------------------------------------------------ runner

def _fold_att(W, a):
    heads, hid = a.shape
    return np.einsum("ihc,hc->ih", W.reshape(W.shape[0], heads, hid), a)


def _ch_perm(heads, hid):
    """Column permutation mapping reference order (h*hid+c) to head-minor
    (c*heads+h): out[:, c*heads+h] = in[:, h*hid+c]."""
    q = np.arange(heads * hid)
    c, h = q // heads, q % heads
    return h * hid + c


class _GatRunner:
    def __init__(self, n_cores=N_CORES):
        self.C = n_cores
        self._graph = None
        self._graph_key = None
        self._kernels = {}

    def graph(self, edge_index, n_nodes):
        key = hash(np.asarray(edge_index).tobytes())
        if key != self._graph_key:
            self._graph = _Graph(edge_index, n_nodes, self.C)
            self._graph_key = key
            self._kernels.clear()
        return self._graph

    def kernel(self, kind, g, *args, bench_loop=1):
        key = (kind, g.T, args, bench_loop)
        if key not in self._kernels:
            if kind == "node":
                self._kernels[key] = _build_node(*args, bench_loop=bench_loop)
            else:
                self._kernels[key] = _build_edge(
                    g.T, g.PC, g.wpc, g.n_groups, *args,
                    bench_loop=bench_loop)
        return self._kernels[key]

    # ---- per-layer helpers (also used by test.py's bench) ----

    def node_inputs(self, g, table_T_f16, wext):
        return [{"xT": np.ascontiguousarray(
                    table_T_f16[:, k * g.shard_nodes:(k + 1) * g.shard_nodes]),
                 "wext": wext} for k in range(self.C)]

    def edge_inputs(self, g, h_table, als, ald, extra_ones):
        iota_v = np.tile(np.arange(P, dtype=np.float16), (P, 1))
        return [{"hsrc": g.stream_h(h_table, k, extra_ones=extra_ones),
                 "zstr": g.stream_z(als, ald, k),
                 "rstr": g.stream_rel(k),
                 "iota": iota_v} for k in range(self.C)]

    def run(self, x, edge_index, W1, a_src1, a_dst1, b1, W2, a_src2, a_dst2,
            b2):
        C = self.C
        N, IN_C = x.shape
        HEADS, HID = a_src1.shape
        HC = HEADS * HID
        OUT_C = W2.shape[1]
        assert not np.any(b1) and not np.any(b2), \
            "nonzero biases need folding into the pass-A tables"
        g = self.graph(edge_index, N)
        cores = list(range(C))

        # ---------------- layer 1
        perm1 = _ch_perm(HEADS, HID)
        wext1 = np.concatenate(
            [np.asarray(W1)[:, perm1], _fold_att(W1, a_src1),
             _fold_att(W1, a_dst1)], axis=1).astype(np.float16)
        xT = np.zeros((IN_C, g.n_win * P), dtype=np.float16)
        xT[:, :N] = np.asarray(x, np.float32).T

        ncA1 = self.kernel("node", g, IN_C, g.wpc, HC + 2 * HEADS, False)
        resA1 = run_bass_kernel_spmd(ncA1, self.node_inputs(g, xT, wext1),
                                     core_ids=cores)
        hext1 = np.concatenate([r["out"] for r in resA1.results], axis=0)

        ncB1 = self.kernel("edge", g, HC, HEADS, HID, False)
        mapsB1 = self.edge_inputs(g, hext1[:, 0:HC],
                                  hext1[:, HC:HC + HEADS],
                                  hext1[:, HC + HEADS:HC + 2 * HEADS], False)
        resB1 = run_bass_kernel_spmd(ncB1, mapsB1, core_ids=cores)
        out1 = np.concatenate([r["out"] for r in resB1.results], axis=0)

        # ---------------- layer 2 (pass A applies ELU to out1)
        W2p = np.asarray(W2)[perm1, :]          # rows to head-minor order
        was2 = _fold_att(W2, a_src2)[perm1]     # [HC, 1]
        wad2 = _fold_att(W2, a_dst2)[perm1]
        wext2 = np.concatenate([W2p, was2, wad2], axis=1).astype(np.float16)
        x2T = np.ascontiguousarray(out1.T)      # [HC, n_win*P] f16

        ncA2 = self.kernel("node", g, HC, g.wpc, OUT_C + 2, True)
        resA2 = run_bass_kernel_spmd(ncA2, self.node_inputs(g, x2T, wext2),
                                     core_ids=cores)
        hext2 = np.concatenate([r["out"] for r in resA2.results], axis=0)

        ncB2 = self.kernel("edge", g, OUT_C, 1, OUT_C, True)
        mapsB2 = self.edge_inputs(g, hext2[:, 0:OUT_C],
                                  hext2[:, OUT_C:OUT_C + 1],
                                  hext2[:, OUT_C + 1:OUT_C + 2], True)
        resB2 = run_bass_kernel_spmd(ncB2, mapsB2, core_ids=cores)
        out2 = np.concatenate([r["out"] for r in resB2.results], axis=0)
        return out2[:N]


_RUNNER = _GatRunner()


def kernel(x, edge_index, W1, a_src1, a_dst1, b1, W2, a_src2, a_dst2, b2):
    """Full-input / full-output entry point. Returns [N, OUT_C] float32."""
    args = [np.asarray(v) for v in
            (x, edge_index, W1, a_src1, a_dst1, b1, W2, a_src2, a_dst2, b2)]
    return _RUNNER.run(*args).astype(np.float32)
